# revision 38
# baseline (speedup 1.0000x reference)
"""Trainium2 Bass kernel for causal multi-head attention block (GPT-style).

Reference computation (fp32):
    qkv = x @ w_attn + b_attn          # [B,S,3E], heads interleaved per 192 cols
    q,k,v per head (d=64), scores = q k^T / 8, causal mask, softmax
    a = softmax @ v ; h = a @ w_proj + b_proj

Sharding (8 cores): core c -> batch b = c//4, head group g = c%4 (4 heads).
Each core computes qkv for its heads, full causal attention, and a partial
c_proj over its 256 e_in rows; bf16 ReduceScatter(add) chunks per batch group
yield each core's slice of the final output. b_proj added on host.

v3 layout/schedule:
  - Phase A runs per 512-token chunk, interleaved with attention query tiles,
    so compute starts as soon as the first xT chunk lands and the first
    ReduceScatter issues ~3x earlier.
  - Scores are bf16 K=64 matmuls on the two 64-row PE groups (h0/h64), which
    the PE executes concurrently -- two heads' score streams overlap.
    The 1/sqrt(d)=1/8 scale folds into the EXP activation scale.
    K-projection bias is dropped entirely (softmax shift-invariance per query).
  - c_proj partials stream to DRAM in bf16; ReduceScatter runs in bf16 in
    halves for overlap and a short tail; the out_d DMAs issue from the gpsimd
    queue so a pending RS never head-of-line blocks the Sync DMA queue (that
    blocking cost the old kernel ~35us).

On-device dataflow per head pair (heads stacked on partition halves):
    QKV psum via PE (bf16, M order per pair = [A-d64 | B-d64])
    S^T[key,q] psum = KT_h^T QT_h (K=64; heads on row groups h0/h64)
    P = exp(S^T/8) via ACT, tri-masked on diag tiles
    a^T|denom psum[65,512] += [V_h|1]^T P  (ones col gives softmax denom)
    at = a^T * recip(denom) broadcast  -> c_proj lhsT [64, tok]
"""

import sys

import numpy as np

if "/opt/trn_rl_repo" not in sys.path:
    sys.path.insert(0, "/opt/trn_rl_repo")

B, S, E, H, D = 2, 2048, 1024, 16, 64
N_CORES = 8
PAIRS = 2  # head pairs per core
ET = 8  # e tiles of 128 over E=1024
QT_N = 4  # query tiles of 512
TT_N = 4  # token tiles of 512
VT_N = 16  # token tiles of 128 (V / c_proj)

_cache = {}


def _build():
    import concourse.mybir as mybir
    import concourse.tile as tile
    from concourse import bacc
    from contextlib import ExitStack

    f32 = mybir.dt.float32
    bf16 = mybir.dt.bfloat16
    ALU = mybir.AluOpType
    AF = mybir.ActivationFunctionType

    nc = bacc.Bacc(
        "TRN2", target_bir_lowering=False, debug=False, num_devices=N_CORES
    )

    xT_d = nc.declare_dram_parameter("xT", [E, S], bf16, isOutput=False)
    wqk_d = nc.declare_dram_parameter(
        "wqk", [128, PAIRS, 2, ET, 128], bf16, isOutput=False
    )
    wv_d = nc.declare_dram_parameter("wv", [128, ET, 256], bf16, isOutput=False)
    bqv_d = nc.declare_dram_parameter("bqv", [128, 2 + 256], f32, isOutput=False)
    trisel_d = nc.declare_dram_parameter("trisel", [128, 384], bf16, isOutput=False)
    wp_d = nc.declare_dram_parameter("wp", [128, PAIRS, 1024], bf16, isOutput=False)
    out_d = nc.declare_dram_parameter("out", [512, 1024], bf16, isOutput=True)

    with ExitStack() as ctx:
        ctx.enter_context(
            nc.allow_low_precision(reason="bf16/fp8 internal math, 2e-2 rel gate")
        )
        tc = ctx.enter_context(tile.TileContext(nc))
        const = ctx.enter_context(tc.tile_pool(name="const", bufs=1))
        dram = ctx.enter_context(tc.tile_pool(name="dram", bufs=1, space="DRAM"))
        psum = ctx.enter_context(tc.tile_pool(name="psum", bufs=4, space="PSUM"))
        psum_av = ctx.enter_context(tc.tile_pool(name="psum_av", bufs=2, space="PSUM"))
        pbuf = ctx.enter_context(tc.tile_pool(name="pbuf", bufs=6))

        # ---- persistent SBUF tensors ----
        xT = const.tile([128, ET, S], bf16, tag="xT")  # 4 MB
        wqk = const.tile([128, PAIRS, 2, ET, 128], bf16, tag="wqk")
        wv = const.tile([128, ET, 256], bf16, tag="wv")
        bqv = const.tile([128, 2 + 256], f32, tag="bqv")
        trisel = const.tile([128, 384], bf16, tag="trisel")
        wp = const.tile([128, PAIRS, 1024], bf16, tag="wp")
        qt_sb = const.tile([128, PAIRS, S], bf16, tag="qt")  # rows 0-63 head A
        kt_sb = const.tile([128, PAIRS, S], bf16, tag="kt")
        vv = const.tile([128, VT_N, 4 * 65], bf16, tag="vv")  # [key,vt,(h,d|1)]
        at = const.tile([128, PAIRS, S], bf16, tag="at")  # pair-stacked a^T

        tri = trisel[:, 0:128]
        sel = trisel.rearrange("p (a b) -> p a b", a=3)[:, 1:3, :]  # [128,2,128]

        # ---- input DMAs: first V-proj needs only xT[:,0,chunk0] + wv ----
        nc.sync.dma_start(out=xT[:, 0, 0:512], in_=xT_d[0:128, 0:512])
        nc.sync.dma_start(out=wv[:], in_=wv_d[:])
        for et in range(1, ET):
            nc.sync.dma_start(
                out=xT[:, et, 0:512], in_=xT_d[et * 128 : (et + 1) * 128, 0:512]
            )
        nc.sync.dma_start(out=bqv[:], in_=bqv_d[:])
        nc.sync.dma_start(out=wqk[:], in_=wqk_d[:])
        nc.sync.dma_start(out=trisel[:], in_=trisel_d[:])
        nc.sync.dma_start(out=wp[:], in_=wp_d[:])
        for tt in range(1, TT_N):
            sl = slice(tt * 512, (tt + 1) * 512)
            for et in range(ET):
                nc.sync.dma_start(
                    out=xT[:, et, sl], in_=xT_d[et * 128 : (et + 1) * 128, sl]
                )
        nc.vector.memset(vv.rearrange("p t (h e) -> p t h e", h=4)[:, :, :, 64:65], 1.0)

        cc_in = []
        for g in range(8):
            cc_in.append(
                dram.tile([256, 1024], bf16, tag=f"cc_in{g}", name=f"cc_in{g}")
            )
        # one RS output tile per 256-token block g (out_d rows g*64..g*64+64)
        cc_outh = {}
        for g in range(8):
            cc_outh[g] = dram.tile(
                [64, 1024], bf16, tag=f"cc_oh{g}", name=f"cc_oh{g}"
            )

        def phase_a_v(tt):
            """V projections for 512-token chunk tt."""
            for vt in range(4 * tt, 4 * tt + 4):
                sl = slice(vt * 128, (vt + 1) * 128)
                ps_v = psum.tile([128, 256], f32, tag="mm", name=f"psv_{vt}")
                for et in range(ET):
                    nc.tensor.matmul(
                        ps_v,
                        lhsT=xT[:, et, sl],
                        rhs=wv[:, et],
                        start=(et == 0),
                        stop=(et == ET - 1),
                    )
                nc.vector.tensor_tensor(
                    out=vv.rearrange("p t (h e) -> p t h e", h=4)[:, vt, :, 0:64],
                    in0=ps_v.rearrange("p (h e) -> p h e", h=4),
                    in1=bqv[:, 2:258].rearrange("p (h e) -> p h e", h=4),
                    op=ALU.add,
                )

        def phase_a_qk(tt):
            """Q/K projections for 512-token chunk tt."""
            sl = slice(tt * 512, (tt + 1) * 512)
            for p in range(PAIRS):
                ps_q = psum.tile([128, 512], f32, tag="mm", name=f"psq_{p}_{tt}")
                for et in range(ET):
                    nc.tensor.matmul(
                        ps_q,
                        lhsT=wqk[:, p, 0, et],
                        rhs=xT[:, et, sl],
                        start=(et == 0),
                        stop=(et == ET - 1),
                    )
                nc.vector.tensor_scalar_add(
                    qt_sb[:, p, sl], ps_q, bqv[:, p : p + 1]
                )
                ps_k = psum.tile([128, 512], f32, tag="mm", name=f"psk_{p}_{tt}")
                for et in range(ET):
                    nc.tensor.matmul(
                        ps_k,
                        lhsT=wqk[:, p, 1, et],
                        rhs=xT[:, et, sl],
                        start=(et == 0),
                        stop=(et == ET - 1),
                    )
                # k bias dropped: per-query-constant shift cancels in softmax
                nc.scalar.copy(kt_sb[:, p, sl], ps_k[:])

        def flush_head(ti, q0, qlen, den4, atu):
            """normalize: recip -> broadcast (sel matmul) -> at mult."""
            rec4 = pbuf.tile([128, qlen], bf16, tag="recb", bufs=2, name=f"rec_{ti}")
            nc.vector.reciprocal(rec4[:], den4[:])
            for pi in range(PAIRS):
                rb = psum.tile([128, qlen], f32, tag="cc", bufs=2, name=f"rb_{ti}_{pi}")
                nc.tensor.matmul(
                    rb, lhsT=sel[:, pi, :], rhs=rec4[:], start=True, stop=True
                )
                nc.vector.tensor_tensor(
                    out=at[:, pi, q0 : q0 + qlen],
                    in0=atu[pi][:],
                    in1=rb[:],
                    op=ALU.mult,
                )

        def flush_cproj(tt_range):
            """c_proj for 128-token tiles in tt_range; RS per 256-token block."""
            for tt in tt_range:
                g = tt // 2  # cc_in block (256 tokens)
                for nt in range(2):
                    ps_c = psum.tile([128, 512], f32, tag="cc", bufs=2)
                    for pi in range(PAIRS):
                        nc.tensor.matmul(
                            ps_c,
                            lhsT=at[:, pi, tt * 128 : (tt + 1) * 128],
                            rhs=wp[:, pi, nt * 512 : (nt + 1) * 512],
                            start=(pi == 0),
                            stop=(pi == PAIRS - 1),
                        )
                    cst = pbuf.tile(
                        [128, 512], bf16, tag="cstage", bufs=3, name=f"cst_{tt}_{nt}"
                    )
                    nc.vector.tensor_copy(out=cst[:], in_=ps_c[:])
                    nc.sync.dma_start(
                        out=cc_in[g][
                            (tt % 2) * 128 : (tt % 2 + 1) * 128,
                            nt * 512 : (nt + 1) * 512,
                        ],
                        in_=cst[:],
                    )
                if tt % 2 == 1:  # a 256-token block g just completed
                    nc.gpsimd.collective_compute(
                        "ReduceScatter",
                        mybir.AluOpType.add,
                        replica_groups=[[0, 1, 2, 3], [4, 5, 6, 7]],
                        ins=[cc_in[g][:].opt()],
                        outs=[cc_outh[g][:].opt()],
                        unique_tensors="Yes",
                    )
                    nc.gpsimd.dma_start(
                        out=out_d[g * 64 : (g + 1) * 64, :],
                        in_=cc_outh[g][:],
                    )

        def attention(ti, q0, qlen):
            """scores -> exp -> AV for queries [q0, q0+qlen); returns (den4, atu)."""
            den4 = pbuf.tile([128, qlen], f32, tag="den", bufs=2, name=f"den_{ti}")
            nc.vector.memset(den4[:], 1.0)
            atu_pair = []
            qtd = q0 // 128  # first diagonal key tile
            nkt = (q0 + qlen) // 128
            for p in range(PAIRS):
                av = []
                for hh in range(2):
                    av.append(
                        psum_av.tile(
                            [65, qlen], f32, tag="av", name=f"av_{p}_{ti}_{hh}"
                        )
                    )
                for kt in range(nkt):
                    j = kt - qtd
                    c0 = 128 * j if j >= 0 else 0
                    n = qlen - c0
                    ps_s = []
                    pts = []
                    for hh in range(2):
                        base = 64 * hh
                        ps_s.append(
                            psum.tile(
                                [128, 512], f32, tag="mm",
                                name=f"ps_s_{p}_{ti}_{kt}_{hh}",
                            )
                        )
                        nc.tensor.matmul(
                            ps_s[hh][:, 0:n],
                            lhsT=kt_sb[
                                base : base + 64, p, kt * 128 : (kt + 1) * 128
                            ],
                            rhs=qt_sb[base : base + 64, p, q0 + c0 : q0 + qlen],
                            start=True,
                            stop=True,
                        )
                    for hh in range(2):
                        pt = pbuf.tile(
                            [128, 512], bf16, tag="p", bufs=8,
                            name=f"pt_{p}_{ti}_{kt}_{hh}",
                        )
                        pts.append(pt)
                        nc.scalar.activation(
                            pt[:, 0:n], ps_s[hh][:, 0:n], AF.Exp, scale=0.125
                        )
                        if j >= 0:
                            nc.vector.tensor_tensor(
                                out=pt[:, 0:128], in0=pt[:, 0:128], in1=tri,
                                op=ALU.mult,
                            )
                    for hh in range(2):
                        h_idx = 2 * p + hh
                        nc.tensor.matmul(
                            av[hh][:, c0:qlen],
                            lhsT=vv[:, kt, h_idx * 65 : (h_idx + 1) * 65],
                            rhs=pts[hh][:, 0:n],
                            start=(kt == 0),
                            stop=(kt == nkt - 1),
                        )
                for hh in range(2):
                    h_idx = 2 * p + hh
                    nc.vector.tensor_copy(
                        out=den4[h_idx * 32 : h_idx * 32 + 1, :], in_=av[hh][64:65, :]
                    )
                atu2 = pbuf.tile(
                    [128, qlen], f32, tag="atu", bufs=4, name=f"atu_{p}_{ti}"
                )
                nc.vector.tensor_copy(out=atu2[0:64, :], in_=av[0][0:64, :])
                nc.vector.tensor_copy(out=atu2[64:128, :], in_=av[1][0:64, :])
                atu_pair.append(atu2)
            return den4, atu_pair

        phase_a_v(0)
        phase_a_qk(0)
        d0 = attention(0, 0, 512)
        phase_a_v(1)
        phase_a_qk(1)
        flush_head(0, 0, 512, *d0)
        flush_cproj(range(0, 4))
        d1 = attention(1, 512, 512)
        phase_a_v(2)
        phase_a_qk(2)
        flush_head(1, 512, 512, *d1)
        flush_cproj(range(4, 8))
        d2 = attention(2, 1024, 512)
        phase_a_v(3)
        phase_a_qk(3)
        flush_head(2, 1024, 512, *d2)
        flush_cproj(range(8, 10))
        d3 = attention(3, 1536, 512)
        flush_cproj(range(10, 12))
        flush_head(3, 1536, 512, *d3)
        flush_cproj(range(12, 16))

    nc.compile()
    return nc


def _prepare_in_maps(x, w_attn, b_attn, w_proj):
    import ml_dtypes

    bf = ml_dtypes.bfloat16
    in_maps = []
    trisel = np.zeros((128, 384), dtype=bf)
    trisel[:, 0:128] = np.triu(np.ones((128, 128), dtype=bf))
    for core in range(N_CORES):
        b, g = core // 4, core % 4
        heads = [4 * g + i for i in range(4)]
        xT = np.ascontiguousarray(x[b].T)  # [1024, 2048]
        wqk_blocks = []
        bq_cols = []
        for pr in range(PAIRS):
            hA, hB = heads[2 * pr], heads[2 * pr + 1]
            qk_pair = []
            for off in (0, 64):  # q cols, k cols
                blk = np.concatenate(
                    [
                        w_attn[:, hA * 192 + off : hA * 192 + off + 64],
                        w_attn[:, hB * 192 + off : hB * 192 + off + 64],
                    ],
                    axis=1,
                )  # [1024, 128] cols (hh, d)
                # [1024,128] -> [128part, 8et, 128]
                qk_pair.append(blk.reshape(ET, 128, 128).transpose(1, 0, 2))
            wqk_blocks.append(np.stack(qk_pair, axis=1))  # [128, 2, 8, 128]
            bq_cols.append(
                np.concatenate(
                    [b_attn[hA * 192 : hA * 192 + 64], b_attn[hB * 192 : hB * 192 + 64]]
                )
            )
        wqk_h = np.stack(wqk_blocks, axis=1)  # [128, 2pair, 2qk, 8, 128]
        wv_blk = np.concatenate(
            [w_attn[:, h * 192 + 128 : h * 192 + 192] for h in heads], axis=1
        )  # [1024, 256]
        wv_h = wv_blk.reshape(ET, 128, 256).transpose(1, 0, 2)  # [128, 8, 256]
        bv_row = np.concatenate(
            [b_attn[h * 192 + 128 : h * 192 + 192] for h in heads]
        )  # [256]
        bqv = np.zeros((128, 258), dtype=np.float32)
        bqv[:, 0] = bq_cols[0]
        bqv[:, 1] = bq_cols[1]
        bqv[:, 2:258] = np.broadcast_to(bv_row, (128, 256))
        wp_h = np.empty((128, PAIRS, 1024), dtype=np.float32)
        ts = trisel.copy()
        for pr in range(PAIRS):
            hA, hB = heads[2 * pr], heads[2 * pr + 1]
            wp_h[0:64, pr, :] = w_proj[hA * 64 : (hA + 1) * 64, :]
            wp_h[64:128, pr, :] = w_proj[hB * 64 : (hB + 1) * 64, :]
            ts[(2 * pr) * 32, 128 + pr * 128 : 128 + pr * 128 + 64] = 1.0
            ts[(2 * pr + 1) * 32, 128 + pr * 128 + 64 : 128 + pr * 128 + 128] = 1.0
        in_maps.append(
            {
                "xT": np.ascontiguousarray(xT.astype(bf)),
                "wqk": np.ascontiguousarray(wqk_h.astype(bf)),
                "wv": np.ascontiguousarray(wv_h.astype(bf)),
                "bqv": bqv,
                "trisel": np.ascontiguousarray(ts),
                "wp": np.ascontiguousarray(wp_h.astype(bf)),
            }
        )
    return in_maps


def _run(x, w_attn, b_attn, w_proj, b_proj, trace=False):
    from concourse.bass_utils import run_bass_kernel_spmd

    if "nc" not in _cache:
        _cache["nc"] = _build()
    nc = _cache["nc"]
    in_maps = _prepare_in_maps(x, w_attn, b_attn, w_proj)
    res = run_bass_kernel_spmd(nc, in_maps, list(range(N_CORES)), trace=trace)
    outs = []
    for b in range(B):
        full = np.empty((S, E), dtype=np.float32)
        for r_ in range(4):
            core_out = np.asarray(res.results[4 * b + r_]["out"], dtype=np.float32)
            for g in range(8):  # 256-token blocks, RS'd per block
                t0 = g * 256 + r_ * 64
                full[t0 : t0 + 64] = core_out[g * 64 : (g + 1) * 64]
        outs.append(full + b_proj[None, :])
    return np.stack(outs).astype(np.float32), res


def kernel(x, w_attn, b_attn, w_proj, b_proj):
    x = np.asarray(x, dtype=np.float32)
    w_attn = np.asarray(w_attn, dtype=np.float32)
    b_attn = np.asarray(b_attn, dtype=np.float32)
    w_proj = np.asarray(w_proj, dtype=np.float32)
    b_proj = np.asarray(b_proj, dtype=np.float32)
    out, _ = _run(x, w_attn, b_attn, w_proj, b_proj, trace=False)
    return out
